# revision 45
# baseline (speedup 1.0000x reference)
"""Multi-head causal attention (B=4, T=2048, D=1024, H=16, Dh=64) on 8 trn2 cores.

Sharding: 4-way DP over batch x 2-way TP over heads.
Core c handles batch c//2 and heads (c%2)*8 .. (c%2)*8+7.
Each core computes a partial output [T, D] (its heads' contribution through
w_out rows); host sums the two partials per batch.

Per-core device kernel (bf16 matmul operands, fp32 PSUM accumulation):
  v[t, f]   = sum_d xT[d, t] * w_v[d, f]      (v in [tok, feat] layout,
                                               + fused ones column per head)
  qkT[f, t] = sum_d w_qk[d, f] * xT[d, t]     (q/k in [feat, tok] layout)
  attention, q-block j OUTER, head-pair hp inner, 2 k-tiles per period:
      S^T[k, q] = sum_d kT[d, k] * qT[d, q]   (row-split PE mode: the two
                                               heads of a pair use disjoint
                                               64-row PE groups, concurrent)
      P^T = exp(S^T / 8)                      (ACT; no max-subtraction)
      causal mask on diagonal k-tiles via gpsimd affine_select
      o^T[m, q] = sum_k v_aug[k, m] * P^T[k, q]   (m: 64 v-feats + ones row
                                                   -> row 64 = denominator)
      attn^T = o^T[0:64] * recip(o^T[64]) broadcast via gpsimd
               partition_broadcast (PE rank-1 matmul for the final pair,
               where the PE is idle and latency is king)
  y[t, n] = sum_f attn^T[f, t] * w_o[f, n]

Scheduling: periods batch TWO k-tiles of S^T (split-mode span) before
switching the PE back to normal mode for filler/PV work -- each
split<->normal transition costs ~100ns of PE drain (the next LDWEIGHTS
must wait for the in-flight matmul of the other row-regime to complete).
Projection groups pop from a deadline-ordered queue (2 periods before
consumption); out-projection groups are slack-scheduled into the late
blocks where the ACT exp throughput (~1ns/col) would otherwise outpace
the lean S^T+PV stream, idle the PE, and make HAM throttle the clock.
The PV pipeline runs 3 k-tiles deep to ride out exp/mask latency.

Startup (the critical ~25us): DRAM layouts are host-repacked so every
transfer has long contiguous per-partition lines; the first wave
(xt tb0 + all weights, 4MB) is split across the three DMA rings
(sync/scalar/gpsimd) at medium granularity -- fine enough to start the
first projection groups early, coarse enough that per-transfer
completion-semaphore latency (~3-4us when loaded) doesn't dominate; each
head-pair's q and k weights ship as ONE transfer so their consumers gate
on a single semaphore. The 4MB of bulk prefetch (xt tb1-3, w_o) is held
out of the critical window by 1-element WAW probe writes: each prefetch
DMA depends on a tile produced at the right period, and the wait lands
on the otherwise-idle sync engine (issuing early from ANY engine makes
the prefetch race the first wave for HBM; delaying via a busy engine
stalls that engine's real work). PE warm-up matmuls bridge the ~8us
engine bootstrap to the first data-dependent work for the HAM clock ramp
(cold PE runs at 1.2GHz; sustained activity unthrottles to 2.4GHz).

Output: y ships as bf16 (halves the 4MB write-back; host sums the two
TP partials in fp32), one 256KB DMA per token-tile.

Known dead ends (tested on HW): running PV in the same 64-row split
regime as S^T via K=64 half-contractions paired across heads passes
CoreSim but wedges the device -- concurrent row-group matmuls that
accumulate into the same PSUM region race on the read-modify-write.
fp8 fails the 2e-2 error budget (~5% expected).
"""

import numpy as np
import ml_dtypes

import concourse.mybir as mybir
import concourse.tile as tile
from concourse import bacc, bass_utils

F32 = mybir.dt.float32
BF16 = mybir.dt.bfloat16

D = 1024          # model dim
T = 2048          # tokens per batch
DH = 64           # head dim
NH_LOC = 8        # heads per core
DT = D // 128     # D tiles (contraction)
TT = T // 128     # token tiles
QB = T // 512     # q blocks of 512
VW = DH + 1       # v width incl ones column
NWARM = 16        # HAM warm-up matmuls

# period index bookkeeping: block j has 4 hps x (j+1) two-kt periods
STARTS = [0, 8, 24, 48]


def hp_start(j, hp):
    return STARTS[j] + hp * 2 * (j + 1)


def build_kernel():
    nc = bacc.Bacc()
    # DRAM layouts are host-side packed so every partition line is a LONG
    # contiguous DRAM run (8KB for xt/wv, 2KB for wqk) -> few, large DMA
    # descriptors -> near-peak HBM bandwidth:
    #  xT:   (tb, p, dt, c) -> per-tb [128, 4096] 1MB blocks, 8KB/partition
    #  w_qk: (f, p, dt, c)  -> per-f [128, 1024] 256KB blocks, 2KB/partition
    #  w_v:  (p, dt, c)     -> one [128, 4096] 1MB block, 8KB/partition
    #  w_o:  (p, hp4, c)    -> one [128, 4096] 1MB block, 8KB/partition
    xT_d = nc.dram_tensor("xT", [512, 4096], BF16, kind="ExternalInput")
    wqk_d = nc.dram_tensor("w_qk", [1024, 1024], BF16, kind="ExternalInput")
    wv_d = nc.dram_tensor("w_v", [128, 4096], BF16, kind="ExternalInput")
    wo_d = nc.dram_tensor("w_o", [128, 4096], BF16, kind="ExternalInput")
    y_d = nc.dram_tensor("y", [T, D], BF16, kind="ExternalOutput")

    with tile.TileContext(nc) as tc:
        with (
            tc.tile_pool(name="big", bufs=1) as big,
            tc.tile_pool(name="ptp", bufs=6) as ptp,
            tc.tile_pool(name="ovp", bufs=8) as ovp,
            tc.tile_pool(name="stg", bufs=2) as stg,
            tc.tile_pool(name="ps_st", bufs=2, space="PSUM") as ps_st,
            tc.tile_pool(name="ps_pv", bufs=2, space="PSUM") as ps_pv,
            tc.tile_pool(name="ps_mm", bufs=2, space="PSUM") as ps_mm,
        ):
            # xt: (tb, dt, c) free layout -- per-tb slice is contiguous
            xt_all = big.tile([128, 4, DT, 512], BF16, tag="xt")
            # wqk: (f, dt, c) free layout -- per-f slice is contiguous
            wqk_all = big.tile([128, 8, DT, 128], BF16, tag="wqk")
            wv_all = big.tile([128, DT, 512], BF16, tag="wv")
            wo_all = big.tile([128, 4, 1024], BF16, tag="wo")
            qk = [big.tile([128, T], BF16, tag=f"qk{i}", name=f"qk{i}") for i in range(8)]
            attn_t = [big.tile([128, T], BF16, tag=f"attn{i}", name=f"attn{i}") for i in range(4)]
            # v operand per head: 64 ONES columns (0:64) then 64 v-feature
            # columns (64:128). PV matmuls are N-bound, so the extra M rows
            # are free -- they deliver the softmax denominator pre-broadcast
            # across 64 PSUM partitions.
            vsb_t = [big.tile([128, 2, NH_LOC * 128], BF16, tag=f"vsb{i}", name=f"vsb{i}") for i in range(8)]
            warm = big.tile([1, 512], BF16, tag="warm")
            vsb_r = [t.rearrange("p t (h c) -> p t h c", c=128) for t in vsb_t]

            # ---- HAM warm-up: PE activity from the end of engine bootstrap
            # (~8us) until the first DMA-fed groups are ready ----
            nc.vector.memset(warm, 1.0)
            for i in range(8):
                nc.vector.memset(vsb_r[i][:, :, :, 0:DH], 1.0)
            ps_w = ps_mm.tile([128, 512], F32, tag="mm")
            for _ in range(NWARM):
                nc.tensor.matmul(ps_w[0:1, 0:256], lhsT=warm[0:1, 0:1],
                                 rhs=warm[0:1, 0:256], start=True, stop=True)

            # ---- DMA emitters ----
            def dma_xt(tb, eng):
                # whole token-block: 1MB, 8KB contiguous per partition
                eng.dma_start(
                    xt_all[:, tb].rearrange("p d c -> p (d c)"),
                    xT_d[tb * 128:(tb + 1) * 128, :],
                )

            def dma_xt_fine(tb, dt, eng):
                # per-dt slice: used for tb0 so the first projection matmuls
                # start as soon as the first 128KB lands
                eng.dma_start(
                    xt_all[:, tb, dt],
                    xT_d[tb * 128:(tb + 1) * 128, dt * 512:(dt + 1) * 512],
                )

            # wqk f-axis order in DRAM/SBUF: q0,k0,q1,k1,q2,k2,q3,k3
            FMAP = {0: 0, 4: 1, 1: 2, 5: 3, 2: 4, 6: 5, 3: 6, 7: 7}

            def dma_wqk_pair(hp, eng):
                # one 512KB transfer for a head-pair's q AND k tiles: both
                # consumers gate on a single completion semaphore (each sem
                # lags its data by several us in the loaded startup window)
                eng.dma_start(
                    wqk_all[:, 2 * hp:2 * hp + 2].rearrange(
                        "p f d c -> p f (d c)"),
                    wqk_d[hp * 256:(hp + 1) * 256, :].rearrange(
                        "(f p) c -> p f c", f=2),
                )

            def dma_xt_span(tb, d0, d1, eng):
                eng.dma_start(
                    xt_all[:, tb, d0:d1].rearrange("p d c -> p (d c)"),
                    xT_d[tb * 128:(tb + 1) * 128, d0 * 512:d1 * 512],
                )

            def dma_wv_span(d0, d1, eng):
                eng.dma_start(
                    wv_all[:, d0:d1].rearrange("p d c -> p (d c)"),
                    wv_d[:, d0 * 512:d1 * 512],
                )

            def dma_wo(eng):
                eng.dma_start(
                    wo_all.rearrange("p h c -> p (h c)"),
                    wo_d[:, :],
                )

            # critical first wave over the three DMA-issuing queues. Each
            # dma_start costs ~0.7us of issue time on its engine and only ~4
            # can be in flight per queue (sem-lane reuse), so only the
            # first-consumed blocks are fine-grained; the rest ship as big
            # high-bandwidth transfers.
            dma_xt_fine(0, 0, nc.sync)
            dma_xt_fine(0, 1, nc.sync)
            dma_xt_span(0, 2, 8, nc.sync)
            dma_wqk_pair(0, nc.scalar)
            dma_wqk_pair(1, nc.scalar)
            dma_wv_span(0, 2, nc.gpsimd)
            dma_wv_span(2, 8, nc.gpsimd)
            dma_wqk_pair(2, nc.gpsimd)
            dma_wqk_pair(3, nc.gpsimd)

            # Bulk prefetches (xt tb1-3, wo: 4MB) must stay OUT of the
            # bandwidth-critical first ~20us -- the three DMA rings share
            # HBM round-robin, so anything racing delays the first-wave
            # weights (measured: f4 arriving at 23us instead of 13us,
            # head-of-line blocking the PE for 7us). Engine-time tricks
            # don't work (idle engines race ahead; busy engines stall their
            # real work on DMA-lane waits). Instead give each prefetch a
            # 1-element WAW dependency: a DVE write into its destination
            # sourced from a tile produced at the right time. Tile then
            # schedules the DMA issue after that producer; the wait sits on
            # the idle sync engine.
            def paced_dma(dep_src, dst_probe, issue):
                # the probe write makes the DMA wait (WAW) for dep_src's
                # producer to COMPLETE; the wait lands on the idle sync
                # engine. Must be emitted after the producer (program order).
                def go():
                    nc.vector.tensor_copy(dst_probe, dep_src)
                    issue()
                return go

            dma_q = [
                (1, paced_dma(qk[1][0:1, 0:1],             # qk(1,0) @ P0
                              xt_all[0:1, 1, 0, 0:1],
                              lambda: dma_xt(1, nc.sync))),
                (3, paced_dma(qk[2][0:1, 0:1],             # qk(2,0) @ P2
                              wo_all[0:1, 0, 0:1],
                              lambda: dma_wo(nc.sync))),
                (7, paced_dma(qk[0][0:1, 512:513],         # qk(0,1) @ P6
                              xt_all[0:1, 2, 0, 0:1],
                              lambda: dma_xt(2, nc.sync))),
                (23, paced_dma(qk[0][0:1, 1024:1025],      # qk(0,2) @ P22
                               xt_all[0:1, 3, 0, 0:1],
                               lambda: dma_xt(3, nc.sync))),
            ]

            # ---- projection group emitters ----
            # warm_i: dt indices after which to squeeze a tiny HAM-warming
            # matmul into wtile (ONLY safe for groups emitted before the
            # first pvA/pvB allocation -- in-order PE writes, see below)
            def v_group(tt, warm_i=()):
                def go():
                    ps = ps_mm.tile([128, 512], F32, tag="mm")
                    for dt in range(DT):
                        nc.tensor.matmul(
                            ps,
                            lhsT=xt_all[:, tt // 4, dt,
                                        (tt % 4) * 128:(tt % 4 + 1) * 128],
                            rhs=wv_all[:, dt, :],
                            start=(dt == 0),
                            stop=(dt == DT - 1),
                        )
                        if dt in warm_i:
                            nc.tensor.matmul(wtile[0:1, 0:256],
                                             lhsT=warm[0:1, 0:1],
                                             rhs=warm[0:1, 0:256],
                                             start=True, stop=True)
                    nc.vector.tensor_copy(
                        vsb_r[tt // 2][:, tt % 2, :, DH:128],
                        ps.rearrange("p (h c) -> p h c", c=DH),
                    )
                return go

            def qk_group(f, tb, warm_i=()):
                def go():
                    ps = ps_mm.tile([128, 512], F32, tag="mm")
                    for dt in range(DT):
                        nc.tensor.matmul(
                            ps,
                            lhsT=wqk_all[:, FMAP[f], dt],
                            rhs=xt_all[:, tb, dt],
                            start=(dt == 0),
                            stop=(dt == DT - 1),
                        )
                        if dt in warm_i:
                            nc.tensor.matmul(wtile[0:1, 0:256],
                                             lhsT=warm[0:1, 0:1],
                                             rhs=warm[0:1, 0:256],
                                             start=True, stop=True)
                    nc.vector.tensor_copy(qk[f][:, tb * 512:(tb + 1) * 512], ps)
                return go

            def out_group(tt, eng=None):
                def go():
                    ysb = stg.tile([128, 1024], BF16, tag="y", bufs=4,
                                   name=f"ysb{tt}")
                    for nb in range(2):
                        ps = ps_mm.tile([128, 512], F32, tag="mm")
                        for hp4 in range(4):
                            nc.tensor.matmul(
                                ps,
                                lhsT=attn_t[hp4][:, tt * 128:(tt + 1) * 128],
                                rhs=wo_all[:, hp4, nb * 512:(nb + 1) * 512],
                                start=(hp4 == 0),
                                stop=(hp4 == 3),
                            )
                        nc.vector.tensor_copy(
                            ysb[:, nb * 512:(nb + 1) * 512], ps)
                    # one 256KB DMA per token-tile: 2KB/partition lines keep
                    # the write stream at high bandwidth (512B lines don't)
                    (eng or nc.sync).dma_start(
                        y_d[tt * 128:(tt + 1) * 128, :], ysb)
                return go

            # ---- deadline filler queue (projection groups) ----
            deadline_q = []
            for tt in (0, 1):
                deadline_q.append((0, v_group(tt)))
            deadline_q.append((0, qk_group(1, 0)))
            deadline_q.append((0, qk_group(5, 0)))
            for tt in (2, 3):
                deadline_q.append((1, v_group(tt)))
            for j in range(QB):
                for hp in range(4):
                    if j == 0 and hp in (0, 1):
                        continue  # upfront / added above
                    dl = hp_start(j, hp) - 2
                    deadline_q.append((dl, qk_group(hp, j)))
                    deadline_q.append((dl, qk_group(4 + hp, j)))
            for tt in range(4, TT):
                jb = tt // 4
                deadline_q.append((STARTS[jb] + tt // 2 - 2, v_group(tt)))
            deadline_q.sort(key=lambda e: e[0])

            slack_q = []          # (earliest_period, fn, block_j)
            out_ready = [False] * QB
            stages = []           # deferred epilogue stages (None = spacer)

            def period_extras(P):
                while dma_q and dma_q[0][0] <= P:
                    dma_q.pop(0)[1]()
                while deadline_q and deadline_q[0][0] <= P:
                    deadline_q.pop(0)[1]()
                npop = 2 if (len(stages) > 3 or P >= 44) else 1
                for _ in range(npop):
                    if stages:
                        s = stages.pop(0)
                        if s is not None:
                            s()
                if slack_q and slack_q[0][0] <= P and out_ready[slack_q[0][2]]:
                    slack_q.pop(0)[1]()

            def push_epilogue(hp, j, pvA, pvB):
                # pv rows 0:64 = denominator (pre-broadcast by the ones
                # columns), rows 64:128 = o. Copy o out (frees PSUM after
                # the reciprocals, which read the denominator rows straight
                # from PSUM); multiply deferred one period. DVE-only.
                ova = ovp.tile([64, 512], BF16, tag="ov", name=f"ova{hp}_{j}")
                ovb = ovp.tile([64, 512], BF16, tag="ov", name=f"ovb{hp}_{j}")
                recA = stg.tile([64, 512], F32, tag="recA", name=f"recA{hp}_{j}")
                recB = stg.tile([64, 512], F32, tag="recB", name=f"recB{hp}_{j}")
                nc.vector.tensor_copy(ova, pvA[DH:128, :])
                nc.vector.tensor_copy(ovb, pvB[DH:128, :])
                nc.vector.reciprocal_approx_fast(out=recA, in_=pvA[0:DH, :])
                nc.vector.reciprocal_approx_fast(out=recB, in_=pvB[0:DH, :])

                jc = slice(j * 512, (j + 1) * 512)

                def stage2():
                    nc.vector.tensor_mul(attn_t[hp][0:64, jc], ova, recA)
                    nc.vector.tensor_mul(attn_t[hp][64:128, jc], ovb, recB)
                    if hp == 3:
                        out_ready[j] = True

                stages.extend([None, stage2])

            # up-front: only what attention period 0 needs. These two groups
            # are DMA-transfer-gated (~10-16us); interleave warm matmuls
            # between their per-dt matmuls so HAM sees sustained PE duty and
            # ramps the clock during the waits. The warm target is a spare
            # ps_pv buffer -- write-only, and PE in-order execution makes the
            # later buffer reuse safe with no semaphore stalls.
            wtile = ps_pv.tile([128, 512], F32, tag="pv")
            for f in (0, 4):
                ps = ps_mm.tile([128, 512], F32, tag="mm")
                for dt in range(DT):
                    nc.tensor.matmul(
                        ps,
                        lhsT=wqk_all[:, FMAP[f], dt],
                        rhs=xt_all[:, 0, dt],
                        start=(dt == 0),
                        stop=(dt == DT - 1),
                    )
                    if dt < 3:
                        nc.tensor.matmul(wtile[0:1, 0:256],
                                         lhsT=warm[0:1, 0:1],
                                         rhs=warm[0:1, 0:256],
                                         start=True, stop=True)
                nc.vector.tensor_copy(qk[f][:, 0:512], ps)

            # ---- attention: q-block j OUTER, head-pair inner, 2 k-tiles
            # per period. The two heads of a pair sit on partitions 0-63 /
            # 64-127 of the same qk tiles, so their K=64 S^T matmuls go to
            # disjoint PE row groups and run concurrently. ----
            P = 0
            pv_queue = []  # (go, epi) -- carried ACROSS block boundaries so
            # the PE keeps PV work in flight through the pipeline restart;
            # a block's epilogue fires when its last PV pops. The new
            # block's accumulators are allocated only after the old block's
            # leftovers (and epilogue ops) are emitted, so the PSUM pool
            # reuse sync sees every reader.
            for j in range(QB):
                for hp in range(4):
                    qTf = qk[hp]
                    kTf = qk[4 + hp]
                    nkt = 4 * (j + 1)
                    pvA = None
                    pvB = None

                    def pv_mms(kt, pt, q0, pvA, pvB, hp=hp, nkt=nkt):
                        def go():
                            nc.tensor.matmul(
                                pvA[:, q0:512],
                                lhsT=vsb_r[kt // 2][:, kt % 2, 2 * hp, :],
                                rhs=pt[:, q0:512],
                                start=(kt == 0), stop=(kt == nkt - 1),
                            )
                            nc.tensor.matmul(
                                pvB[:, q0:512],
                                lhsT=vsb_r[kt // 2][:, kt % 2, 2 * hp + 1, :],
                                rhs=pt[:, 512 + q0:1024],
                                start=(kt == 0), stop=(kt == nkt - 1),
                            )
                        return go

                    def emit_st(kt, j=j, hp=hp, qTf=qTf, kTf=kTf):
                        # diagonal k-tiles: q < 128*(kt-4j) is fully masked --
                        # narrow S^T/exp/mask/PV to the live columns
                        q0 = 128 * (kt - 4 * j) if kt >= 4 * j else 0
                        nq = 512 - q0
                        st = ps_st.tile([128, 1024], F32, tag="st")
                        nc.tensor.matmul(
                            st[:, q0:512],
                            lhsT=kTf[0:64, kt * 128:(kt + 1) * 128],
                            rhs=qTf[0:64, j * 512 + q0:(j + 1) * 512],
                            start=True, stop=True,
                        )
                        nc.tensor.matmul(
                            st[:, 512 + q0:1024],
                            lhsT=kTf[64:128, kt * 128:(kt + 1) * 128],
                            rhs=qTf[64:128, j * 512 + q0:(j + 1) * 512],
                            start=True, stop=True,
                        )
                        pt = ptp.tile([128, 1024], BF16, tag="pt",
                                      name=f"pt{hp}_{j}_{kt}")
                        st_r = st.rearrange("p (h q) -> p h q", h=2)
                        pt_r = pt.rearrange("p (h q) -> p h q", h=2)
                        nc.scalar.activation(
                            pt_r[:, :, q0:512], st_r[:, :, q0:512],
                            mybir.ActivationFunctionType.Exp, scale=0.125
                        )
                        if kt >= 4 * j:  # diagonal k-tile: zero where k > q.
                            # Only the first 128 live columns can have masked
                            # elements (k > q requires q - q0 < 128); beyond
                            # them the select keeps everything, so narrowing
                            # the op cuts its latency off the pt->PV chain.
                            for half in range(2):
                                nc.gpsimd.affine_select(
                                    out=pt[:, half * 512 + q0:
                                           half * 512 + q0 + 128],
                                    in_=pt[:, half * 512 + q0:
                                           half * 512 + q0 + 128],
                                    compare_op=mybir.AluOpType.is_ge,
                                    fill=0.0,
                                    base=0,
                                    pattern=[[1, 128]],
                                    channel_multiplier=-1,
                                )
                        return pt, q0

                    for kp in range(nkt // 2):
                        period_extras(P)
                        sts = [emit_st(kt) for kt in (2 * kp, 2 * kp + 1)]
                        if kp == 0:
                            # drain the previous block's tail PVs (they run
                            # after this block's first S^T pair -- no PE
                            # bubble at the boundary) + its epilogue, then
                            # allocate this block's accumulators
                            while pv_queue:
                                go, epi = pv_queue.pop(0)
                                go()
                                if epi is not None:
                                    epi()
                            pvA = ps_pv.tile([128, 512], F32, tag="pv")
                            pvB = ps_pv.tile([128, 512], F32, tag="pv")
                        for kt, (pt, q0) in zip((2 * kp, 2 * kp + 1), sts):
                            epi = None
                            if kt == nkt - 1 and not (j == 3 and hp == 3):
                                epi = (lambda hp=hp, j=j, A=pvA, B=pvB:
                                       push_epilogue(hp, j, A, B))
                            pv_queue.append(
                                (pv_mms(kt, pt, q0, pvA, pvB), epi))
                        while len(pv_queue) > 3:
                            go, epi = pv_queue.pop(0)
                            go()
                            if epi is not None:
                                epi()
                        P += 1

                    if j == 3 and hp == 3:
                        # flush this block's tail PVs, drain pending
                        # epilogues; start the final pair's reciprocals
                        # immediately; fill their latency window with
                        # held-back out groups (keeps the PE busy so HAM
                        # doesn't throttle); then multiply and the last
                        # out-projections
                        while pv_queue:
                            go, _ = pv_queue.pop(0)
                            go()
                        while stages:
                            s = stages.pop(0)
                            if s is not None:
                                s()
                        # final pair: reciprocals from PSUM immediately, no
                        # ov copies (the multiplies read PSUM directly --
                        # nothing needs the banks afterwards); held-back out
                        # groups keep the PE busy through the chain so HAM
                        # doesn't throttle the last out-projections
                        recA = stg.tile([64, 512], F32, tag="recA")
                        recB = stg.tile([64, 512], F32, tag="recB")
                        nc.vector.reciprocal_approx_fast(out=recA,
                                                         in_=pvA[0:DH, :])
                        nc.vector.reciprocal_approx_fast(out=recB,
                                                         in_=pvB[0:DH, :])
                        for _, fn, _ in slack_q:
                            fn()
                        slack_q.clear()
                        jc3 = slice(3 * 512, 4 * 512)
                        nc.vector.tensor_mul(attn_t[3][0:64, jc3],
                                             pvA[DH:128, :], recA)
                        nc.vector.tensor_mul(attn_t[3][64:128, jc3],
                                             pvB[DH:128, :], recB)
                        out_ready[3] = True
                    if hp == 3:
                        if j < 3:
                            # out(2)'s last 3 groups are held for the final
                            # chain window (earliest=999 keeps them queued)
                            earliest = {0: 12, 1: 44, 2: 60}[j]
                            for i, tt in enumerate(range(4 * j, 4 * j + 4)):
                                e = earliest + 4 * i
                                if j == 2 and i >= 1:
                                    e = 999
                                slack_q.append((e, out_group(tt), j))
                        else:
                            for tt in range(12, 16):
                                out_group(tt)()

            while stages:
                s = stages.pop(0)
                if s is not None:
                    s()
            while slack_q:
                slack_q.pop(0)[1]()

    nc.compile()
    return nc


def _shard_inputs(x, w_qkv, w_out):
    """Build the 8 per-core input maps (bf16, DMA-block-packed layouts)."""
    bf16 = ml_dtypes.bfloat16
    in_maps = []
    for c in range(8):
        b = c // 2
        hg = c % 2
        q_cols = slice(hg * 512, hg * 512 + 512)
        k_cols = slice(1024 + hg * 512, 1024 + hg * 512 + 512)
        v_cols = slice(2048 + hg * 512, 2048 + hg * 512 + 512)

        xT = np.ascontiguousarray(x[b].T)                    # [1024, 2048]
        # (tb, p, dt, c): per-tb 1MB blocks, 8KB contiguous per partition
        x_pack = (xT.reshape(8, 128, 4, 512)                 # (dt,p,tb,c)
                  .transpose(2, 1, 0, 3).reshape(512, 4096))

        w_qk = np.concatenate([w_qkv[:, q_cols], w_qkv[:, k_cols]], axis=1)
        # (f, p, dt, c) with f-axis order (0,4,1,5,2,6,3,7): each q/k pair of
        # a head-pair is CONTIGUOUS, so one 512KB DMA covers both and the
        # consumers wait on a single completion semaphore
        wqk_pack = (w_qk.reshape(8, 128, 8, 128)             # (dt,p,f,c)
                    .transpose(2, 1, 0, 3)                   # (f,p,dt,c)
                    [[0, 4, 1, 5, 2, 6, 3, 7]].reshape(1024, 1024))

        # (p, dt, c): one 1MB block, 8KB contiguous per partition
        wv_pack = (w_qkv[:, v_cols].reshape(8, 128, 512)     # (dt,p,c)
                   .transpose(1, 0, 2).reshape(128, 4096))

        # (p, hp4, c): one 1MB block, 8KB contiguous per partition
        wo_pack = (w_out[hg * 512:hg * 512 + 512, :]
                   .reshape(4, 128, 1024)                    # (hp4,p,c)
                   .transpose(1, 0, 2).reshape(128, 4096))

        in_maps.append({
            "xT": np.ascontiguousarray(x_pack).astype(bf16),
            "w_qk": np.ascontiguousarray(wqk_pack).astype(bf16),
            "w_v": np.ascontiguousarray(wv_pack).astype(bf16),
            "w_o": np.ascontiguousarray(wo_pack).astype(bf16),
        })
    return in_maps


def _run(inputs, trace=False):
    x = np.asarray(inputs["x"], dtype=np.float32)
    w_qkv = np.asarray(inputs["w_qkv"], dtype=np.float32)
    w_out = np.asarray(inputs["w_out"], dtype=np.float32)
    nc = build_kernel()
    in_maps = _shard_inputs(x, w_qkv, w_out)
    res = None
    for attempt in range(3):
        try:
            res = bass_utils.run_bass_kernel_spmd(
                nc, in_maps, core_ids=list(range(8)), trace=trace
            )
            break
        except Exception:
            if attempt == 2:
                raise
    assert res is not None
    out = np.empty((4, T, D), dtype=np.float32)
    for b in range(4):
        out[b] = (res.results[2 * b]["y"].astype(np.float32)
                  + res.results[2 * b + 1]["y"].astype(np.float32))
    return out, res


def kernel(**inputs):
    out, _ = _run(inputs, trace=False)
    return out

